# revision 12
# baseline (speedup 1.0000x reference)
"""Trainium2 Bass kernel for CausalSelfAttention2D.

Math (per batch element b):
  xn = ChannelLayerNorm(x)          # over C per spatial position
  qkv = qkv_w @ xn + qkv_b          # 1x1 conv == matmul over C
  per head h: S = (q_h^T k_h)/8 ; causal mask ; P = softmax(S)
  O_h = v_h @ P^T ; out = proj_w @ concat(O) + proj_b
Sharding: data-parallel over B (8 batch elements -> 8 cores).

Host-side algebraic folds (exact):
  - ln_g folded into qkv_w columns; ln_b folded into qkv_b.
  - k-bias dropped entirely (additive f(i) term in scores, softmax no-op).
  - v-part of qkv bias folded into proj_b (softmax rows sum to 1).
  - pos_h/pos_w additive per-head scalar bias is a softmax no-op; dropped.

LN folding (the key restructure vs the straightforward version): instead
of materializing xn and then qkv = W @ xn, compute Z = W @ x on raw x
(starts as soon as x lands, no stats dependency) and fold the LN affine
into each PSUM accumulation group:
  q[o,l] = isd_l*Z[o,l] - (mu_l/sd_l)*r[o] + b[o]
         = isd_l * ( Z[o,l] + (-r[o]*mu_l + b[o]*sd_l) )
where r[o] = sum_c W[o,c]. The parenthesized correction is rank-2 ->
one K=2 matmul per group (lhsT = [-r; b] from host, rhs = [mu; sd] rows
from the on-device stats), and the isd_l scale rides the PSUM->SBUF
drain (one DVE tensor_tensor with a broadcast isd tile).

On-chip layout (per core):
  x:      [C=512, L=1024] as 4 tiles of [128, 1024] (C on partitions)
  LN stats via ones-matmul column sums; [128, 8]-layout scalar chain.
  q, k:   [512, L] 4 tiles [128, 1024]
  vT:     [L, 512] 8 tiles [128, 1024] fp16: [64 v | 64 ones] per head
          so one [128,128] stationary computes AV (rows 0-63) and the
          softmax denominator broadcast (rows 64-127) in a single matmul.
  scores: computed transposed, S^T[j, i], per head pair (row-packed
          K=64 matmuls via tile_position); exp on ACT (scale=1/8) out of
          PSUM into fp16 P^T tiles; causal mask applied post-exp as a
          0/1 triangular multiply on the diagonal 128-col block (GpSimd).
  attention pairs are software-pipelined (scores(p+1) ahead of AV(p)).
  proj:   [512, 512] @ O.
"""

import numpy as np

import concourse.bass as bass
import concourse.mybir as mybir
import concourse.tile as tile
from concourse import bacc
from concourse.bass import ds, ts
from concourse.bass_utils import run_bass_kernel_spmd


F32 = mybir.dt.float32
FP16 = mybir.dt.float16

B, C, H, W = 8, 512, 32, 32
L = H * W                      # 1024
HEADS = 8
DM = 512
DH = 64                        # d_head
EPS = 1e-5
NCORES = 8

# scores^T chunking per j-tile t: list of (i_start, n_cols); each chunk
# stays inside one 512-col PSUM bank of the per-head mega region.
ST_CHUNKS = {
    0: [(0, 512), (512, 512)],
    1: [(128, 512), (640, 384)],
    2: [(256, 512), (768, 256)],
    3: [(384, 512), (896, 128)],
    4: [(512, 512)],
    5: [(640, 384)],
    6: [(768, 256)],
    7: [(896, 128)],
}
ST_EXT = {t: chunks[-1][0] + chunks[-1][1] - 128 * t for t, chunks in ST_CHUNKS.items()}


def _emit(nc, tc):
    x_d = nc.dram_tensor("x", [C, L], FP16, kind="ExternalInput").ap()
    wqkvT_d = nc.dram_tensor("wqkvT", [C, 3 * DM], FP16, kind="ExternalInput").ap()
    wprojT_d = nc.dram_tensor("wprojT", [DM, C], FP16, kind="ExternalInput").ap()
    fix_d = nc.dram_tensor("fix", [2, 3 * DM], FP16, kind="ExternalInput").ap()
    onescol_d = nc.dram_tensor("onescol", [128, 1], FP16, kind="ExternalInput").ap()
    onesrow_d = nc.dram_tensor("onesrow", [1, 128], FP16, kind="ExternalInput").ap()
    bproj_d = nc.dram_tensor("bproj", [C], F32, kind="ExternalInput").ap()
    y_d = nc.dram_tensor("y", [C, L], F32, kind="ExternalOutput").ap()

    fexp = mybir.ActivationFunctionType.Exp
    fsqrt = mybir.ActivationFunctionType.Sqrt
    fcopy = mybir.ActivationFunctionType.Copy

    with (
        tc.tile_pool(name="const", bufs=1) as cpool,
        tc.tile_pool(name="pers", bufs=1) as pers,
        tc.tile_pool(name="pT", bufs=17) as ppool,
    ):
        # ======== persistent tiles ========
        q_t = [pers.tile([128, L], FP16, tag=f"q{m}", name=f"q{m}") for m in range(4)]
        k_t = [pers.tile([128, L], FP16, tag=f"k{m}", name=f"k{m}") for m in range(4)]
        vT_t = [pers.tile([128, 2 * DM], FP16, tag=f"vT{m}", name=f"vT{m}") for m in range(8)]
        o_t = [pers.tile([128, L], FP16, tag=f"o{m}", name=f"o{m}") for m in range(4)]
        wproj_t = [pers.tile([128, C], FP16, tag=f"wp{m}", name=f"wp{m}") for m in range(4)]
        x_t = [pers.tile([128, L], FP16, tag=f"x{c}", name=f"x{c}") for c in range(4)]
        w_t = [pers.tile([128, 3 * DM], FP16, tag=f"w{c}", name=f"w{c}") for c in range(4)]
        fix_t = pers.tile([2, 3 * DM], FP16, tag="fix", name="fix")
        bp4 = pers.tile([128, 4], F32, tag="bp4", name="bp4")
        musd = pers.tile([2, L], FP16, tag="musd", name="musd")
        s_row = pers.tile([1, L], FP16, tag="srow", name="srow")
        s_row32 = pers.tile([1, L], F32, tag="srow32", name="srow32")
        s_col = pers.tile([128, 8], F32, tag="scol", name="scol")
        bs_t = pers.tile([128, L], FP16, tag="bs", name="bs")

        # ======== input DMAs, spread across queues ========
        ones_col = cpool.tile([128, 1], FP16, tag="ones_col")
        ones_row = cpool.tile([1, 128], FP16, tag="ones_row")
        eps128 = cpool.tile([128, 1], F32, tag="eps")
        tri = cpool.tile([128, 128], FP16, tag="tri")

        nc.sync.dma_start(ones_col[:], onescol_d[:])
        nc.sync.dma_start(ones_row[:], onesrow_d[:])
        nc.sync.dma_start(x_t[0][:], x_d[ts(0, 128), :])
        nc.sync.dma_start(x_t[1][:], x_d[ts(1, 128), :])
        nc.gpsimd.dma_start(x_t[2][:], x_d[ts(2, 128), :])
        nc.gpsimd.dma_start(x_t[3][:], x_d[ts(3, 128), :])
        for c in range(4):
            nc.scalar.dma_start(w_t[c][:], wqkvT_d[ts(c, 128), :])
        nc.scalar.dma_start(fix_t[:], fix_d[:])
        for m in range(4):
            nc.gpsimd.dma_start(wproj_t[m][:], wprojT_d[ts(m, 128), :])
        nc.gpsimd.dma_start(bp4[:], bproj_d[:].rearrange("(o p) -> p o", p=128))

        # consts built on engines (emitted after DMAs so queue slots go to
        # DMAs first; these have no deps and fill idle time)
        nc.gpsimd.memset(eps128[:], EPS)
        nc.gpsimd.memset(tri[:], 1.0)
        # tri[p, f] = 1.0 if f >= p else 0.0   (keep i_rel >= j_rel)
        nc.gpsimd.affine_select(
            out=tri[:], in_=tri[:],
            compare_op=mybir.AluOpType.is_ge,
            fill=0.0, base=0, pattern=[[1, 128]], channel_multiplier=-1,
        )
        # vT ones columns: whole-tile memset; the V drains later scatter
        # the v values into the even 64-col groups, ones survive elsewhere
        for m8 in range(8):
            nc.vector.memset(vT_t[m8][:], 1.0)

        # ======== stats + qkv head phase ========
        # Stack order puts psStat+psBC (freed early) at low banks where the
        # attention megas will land, and psB (freed at qkv end) at the high
        # banks that the late-starting psAV pool reuses.
        psStat = tc.alloc_tile_pool(name="psStat", bufs=1, space="PSUM")
        psBC = tc.alloc_tile_pool(name="psBC", bufs=1, space="PSUM")
        psB = tc.alloc_tile_pool(name="psB", bufs=2, space="PSUM")

        # column sums of x and x^2 -> [1, 2048] stats psum
        # layout: [sum ch0 | sum ch1 | sq ch0 | sq ch1]
        stat_all = psStat.tile([1, 2048], F32, tag="stat", name="stat_all")
        sq_t = []
        for c in range(4):
            sq = pers.tile([128, L], FP16, tag=f"sq{c}", name=f"sq{c}")
            nc.vector.tensor_mul(sq[:], x_t[c][:], x_t[c][:])
            sq_t.append(sq)
        for ch in range(2):
            for c in range(4):
                nc.tensor.matmul(
                    stat_all[:, ds(512 * ch, 512)], ones_col[:],
                    x_t[c][:, ts(ch, 512)],
                    start=(c == 0), stop=(c == 3),
                )
        for ch in range(2):
            for c in range(4):
                nc.tensor.matmul(
                    stat_all[:, ds(1024 + 512 * ch, 512)], ones_col[:],
                    sq_t[c][:, ts(ch, 512)],
                    start=(c == 0), stop=(c == 3),
                )

        # PE warmup during the DMA head: keeps the HAM clock-gate open.
        bc_ps = psBC.tile([128, 1024], F32, tag="bc", name="bc_ps")
        ones_den = cpool.tile([128, 64], FP16, tag="ones_den")
        nc.gpsimd.memset(ones_den[:], 1.0)

        def warmup(n):
            for _ in range(n):
                nc.tensor.matmul(bc_ps[ds(0, 64), ds(0, 64)], ones_den[:],
                                 ones_den[:], start=True, stop=True)

        warmup(56)

        # stats chain in [128, 8] layout (1-partition DVE ops are ~100x
        # slower per element; bounce via ACT copy + SBUF DMA reshape).
        stats_sb = pers.tile([1, 2048], F32, tag="statsb", name="stats_sb")
        nc.scalar.activation(stats_sb[:, ds(0, 1024)], stat_all[:, ds(0, 1024)],
                             fcopy, scale=1.0 / C)
        nc.scalar.activation(stats_sb[:, ds(1024, 1024)], stat_all[:, ds(1024, 1024)],
                             fcopy, scale=1.0 / C)
        st = pers.tile([128, 16], F32, tag="st", name="st")   # mu cols 0-7, msq 8-15
        nc.sync.dma_start(st[ds(0, 64), ds(0, 8)], stats_sb[:, ds(0, 512)])
        nc.sync.dma_start(st[ds(64, 64), ds(0, 8)], stats_sb[:, ds(512, 512)])
        nc.scalar.dma_start(st[ds(0, 64), ds(8, 8)], stats_sb[:, ds(1024, 512)])
        nc.scalar.dma_start(st[ds(64, 64), ds(8, 8)], stats_sb[:, ds(1536, 512)])
        mu2 = pers.tile([128, 8], F32, tag="mu2", name="mu2")
        nc.vector.tensor_mul(mu2[:], st[:, ds(0, 8)], st[:, ds(0, 8)])
        nc.vector.tensor_sub(mu2[:], st[:, ds(8, 8)], mu2[:])       # var
        sd_t = pers.tile([128, 8], F32, tag="sd", name="sd")
        nc.scalar.activation(sd_t[:], mu2[:], fsqrt, bias=eps128[:])
        isd_t = pers.tile([128, 8], F32, tag="isd", name="isd")
        nc.vector.reciprocal_approx_fast(isd_t[:], sd_t[:])
        s16 = pers.tile([128, 24], FP16, tag="s16", name="s16")
        nc.vector.tensor_copy(s16[:, ds(0, 8)], st[:, ds(0, 8)])    # mu
        nc.vector.tensor_copy(s16[:, ds(8, 8)], sd_t[:])            # sd
        nc.vector.tensor_copy(s16[:, ds(16, 8)], isd_t[:])          # 1/sd
        nc.sync.dma_start(musd[ds(0, 1), :], s16[:, ds(0, 8)])
        nc.scalar.dma_start(musd[ds(1, 1), :], s16[:, ds(8, 8)])
        nc.sync.dma_start(s_row[ds(0, 1), :], s16[:, ds(16, 8)])
        nc.scalar.dma_start(s_row32[ds(0, 1), :], isd_t[:])
        for j in range(8):
            eng = nc.sync if j % 2 == 0 else nc.scalar
            eng.dma_start(s_col[:, ds(j, 1)], s_row32[ds(0, 1), ts(j, 128)])

        # broadcast isd down 128 partitions via K=1 matmul
        for ch in range(2):
            nc.tensor.matmul(bc_ps[:, ts(ch, 512)], ones_row[:],
                             s_row[:, ts(ch, 512)], start=True, stop=True)
            nc.vector.tensor_copy(bs_t[:, ts(ch, 512)], bc_ps[:, ts(ch, 512)])

        # ---- qkv groups: Z = W @ x  (+ K=2 LN/bias fixup), drain scaled ----
        def qk_group(m, name_off, dst):
            for ch in range(2):
                ps = psB.tile([128, 512], F32, tag="mm")
                for c in range(4):
                    nc.tensor.matmul(
                        ps[:], w_t[c][:, ds(name_off + m * 128, 128)],
                        x_t[c][:, ts(ch, 512)],
                        start=(c == 0), stop=False,
                    )
                nc.tensor.matmul(
                    ps[:], fix_t[:, ds(name_off + m * 128, 128)],
                    musd[:, ts(ch, 512)],
                    start=False, stop=True,
                )
                nc.vector.tensor_mul(dst[:, ts(ch, 512)], ps[:], bs_t[:, ts(ch, 512)])

        def v_group(m8):
            ps = psB.tile([128, 512], F32, tag="mm")
            for c in range(4):
                nc.tensor.matmul(
                    ps[:], x_t[c][:, ts(m8, 128)], w_t[c][:, ds(2 * DM, DM)],
                    start=(c == 0), stop=False,
                )
            nc.tensor.matmul(
                ps[:], musd[ds(0, 1), ts(m8, 128)], fix_t[ds(0, 1), ds(2 * DM, DM)],
                start=False, stop=True,
            )
            # strided drain: scatter v into [64 v | 64 ones] head slots with
            # the per-position isd scale applied
            nc.vector.tensor_scalar_mul(
                vT_t[m8][:, :].rearrange("p (h o) -> p h o", o=128)[:, :, ds(0, 64)],
                ps[:, :].rearrange("p (h o) -> p h o", o=64),
                s_col[:, ds(m8, 1)],
            )

        qk_group(0, 0, q_t[0])
        qk_group(0, DM, k_t[0])
        # priority mark: attention instructions are later re-prioritized to
        # land here so the scheduler interleaves them with the rest of qkv
        p_mark = tc.cur_priority
        for m in (1, 2, 3):
            qk_group(m, 0, q_t[m])
            qk_group(m, DM, k_t[m])
        for m8 in range(8):
            v_group(m8)

        psB.release()
        psBC.release()
        psStat.release()

        # ======== attention (software-pipelined with remaining qkv) ========
        pT_pairs = {}

        def emit_scores(p, psT):
            pT_tiles = {}
            for t in range(8):
                ext = ST_EXT[t]
                i0 = 128 * t
                pT = ppool.tile([128, 2048], FP16, tag="pT")
                megas = []
                for hh in range(2):
                    megas.append(psT.tile([128, 1024], F32, tag="sT",
                                          name=f"sT{p}_{t}_{hh}"))
                for (ist, ncols) in ST_CHUNKS[t]:
                    for hh in range(2):
                        pb = 64 * hh
                        nc.tensor.matmul(
                            megas[hh][:, ds(ist - i0, ncols)],
                            k_t[p][ds(pb, 64), ts(t, 128)],
                            q_t[p][ds(pb, 64), ds(ist, ncols)],
                            start=True, stop=True,
                            tile_position=(pb, 0),
                        )
                for hh in range(2):
                    nc.scalar.activation(
                        pT[:, ds(hh * 1024, ext)],
                        megas[hh][:, ds(0, ext)],
                        fexp, scale=0.125,
                    )
                    nc.gpsimd.tensor_mul(
                        pT[:, ds(hh * 1024, 128)], pT[:, ds(hh * 1024, 128)], tri[:]
                    )
                pT_tiles[t] = pT
            pT_pairs[p] = pT_tiles

        def emit_av(p, psAV, rsb):
            pT_tiles = pT_pairs.pop(p)
            for cch in range(2):
                tlist = range(4) if cch == 0 else range(8)
                avs = []
                for hh in range(2):
                    h = 2 * p + hh
                    av = psAV.tile([128, 512], F32, tag="av",
                                   name=f"av{p}_{cch}_{hh}")
                    avs.append(av)
                    for ti, t in enumerate(tlist):
                        lo = max(cch * 512, 128 * t)
                        n = (cch + 1) * 512 - lo
                        nc.tensor.matmul(
                            av[:, ds(lo - cch * 512, n)],
                            vT_t[t][:, ds(128 * h, 128)],
                            pT_tiles[t][:, ds(hh * 1024 + lo - 128 * t, n)],
                            start=(ti == 0), stop=(ti == len(tlist) - 1),
                        )
                for hh in range(2):
                    rec = rsb.tile([128, 512], F32, tag="rec")
                    nc.vector.reciprocal_approx_fast(rec[:], avs[hh][:, :])
                    nc.vector.tensor_mul(
                        o_t[p][ds(64 * hh, 64), ts(cch, 512)],
                        avs[hh][ds(0, 64), :], rec[ds(64, 64), :],
                    )

        with (
            tc.tile_pool(name="psT", bufs=3, space="PSUM") as psT,
            tc.tile_pool(name="psAV", bufs=2, space="PSUM") as psAV,
            tc.tile_pool(name="rsb", bufs=2) as rsb,
            tc.high_priority(offset=tc.cur_priority - p_mark),
        ):
            emit_scores(0, psT)
            emit_scores(1, psT)
            emit_av(0, psAV, rsb)
            emit_scores(2, psT)
            emit_av(1, psAV, rsb)
            emit_scores(3, psT)
            emit_av(2, psAV, rsb)
            emit_av(3, psAV, rsb)

            # ======== output projection ========
            yq = [nc.sync, nc.gpsimd, nc.scalar, nc.sync]
            for m in range(4):
                yt = rsb.tile([128, L], F32, tag="y")
                for ch in range(2):
                    ps = psAV.tile([128, 512], F32, tag="av")
                    for c2 in range(4):
                        nc.tensor.matmul(
                            ps[:], wproj_t[c2][:, ts(m, 128)],
                            o_t[c2][:, ts(ch, 512)],
                            start=(c2 == 0), stop=(c2 == 3),
                        )
                    nc.vector.tensor_scalar_add(yt[:, ts(ch, 512)], ps[:],
                                                bp4[:, ds(m, 1)])
                yq[m].dma_start(y_d[ts(m, 128), :], yt[:])


_NC_CACHE = None


def build_nc():
    global _NC_CACHE
    if _NC_CACHE is None:
        nc = bacc.Bacc("TRN2", target_bir_lowering=False, debug=False)
        with tile.TileContext(nc) as tc:
            _emit(nc, tc)
        nc.compile()
        _NC_CACHE = nc
    return _NC_CACHE


def host_inputs(x, ln_g, ln_b, qkv_w, qkv_b, proj_w, proj_b, pos_h, pos_w):
    """Fold LN affine + biases; build per-core input maps."""
    x = np.asarray(x, np.float32)
    ln_g = np.asarray(ln_g, np.float32)
    ln_b = np.asarray(ln_b, np.float32)
    qkv_w = np.asarray(qkv_w, np.float32)
    qkv_b = np.asarray(qkv_b, np.float32)
    proj_w = np.asarray(proj_w, np.float32)
    proj_b = np.asarray(proj_b, np.float32)

    w_eff = qkv_w * ln_g[None, :]                    # [1536, 512]
    b_eff = qkv_b + qkv_w @ ln_b                     # [1536]
    wqkvT = np.ascontiguousarray(w_eff.T)            # [512, 1536]
    bq, bv = b_eff[:DM], b_eff[2 * DM:]
    bproj = proj_b + proj_w @ bv                     # [512]
    wprojT = np.ascontiguousarray(proj_w.T)          # [512, 512]

    r = w_eff.sum(axis=1)                            # [1536] row sums
    fix = np.zeros((2, 3 * DM), np.float32)
    fix[0, :] = -r
    fix[1, :DM] = bq                                 # k-bias dropped, v folded

    common = {
        "wqkvT": wqkvT.astype(np.float16),
        "wprojT": wprojT.astype(np.float16),
        "fix": fix.astype(np.float16),
        "bproj": np.ascontiguousarray(bproj),
        "onescol": np.ones((128, 1), np.float16),
        "onesrow": np.ones((1, 128), np.float16),
    }
    in_maps = []
    for b in range(B):
        m = dict(common)
        m["x"] = np.ascontiguousarray(x[b].reshape(C, L)).astype(np.float16)
        in_maps.append(m)
    return in_maps


def kernel(x, ln_g, ln_b, qkv_w, qkv_b, proj_w, proj_b, pos_h, pos_w, **kw):
    nc = build_nc()
    in_maps = host_inputs(x, ln_g, ln_b, qkv_w, qkv_b, proj_w, proj_b, pos_h, pos_w)
    res = run_bass_kernel_spmd(nc, in_maps, core_ids=list(range(NCORES)))
    out = np.stack([res.results[b]["y"].reshape(C, H, W) for b in range(B)])
    return out.astype(np.float32)


if __name__ == "__main__":
    nc = build_nc()
    print("built + compiled ok")


# revision 26
# speedup vs baseline: 1.2842x; 1.2842x over previous
"""Trainium2 Bass kernel for CausalSelfAttention2D.

Math (per batch element b):
  xn = ChannelLayerNorm(x)          # over C per spatial position
  qkv = qkv_w @ xn + qkv_b          # 1x1 conv == matmul over C
  per head h: S = (q_h^T k_h)/8 ; causal mask ; P = softmax(S)
  O_h = v_h @ P^T ; out = proj_w @ concat(O) + proj_b
Sharding: data-parallel over B (8 batch elements -> 8 cores).

Host-side algebraic folds (exact):
  - ln_g folded into qkv_w columns; ln_b folded into qkv_b.
  - k-bias dropped entirely (additive f(i) term in scores, softmax no-op).
  - v-part of qkv bias folded into proj_b (softmax rows sum to 1).
  - pos_h/pos_w additive per-head scalar bias is a softmax no-op; dropped.

LN folding (the key restructure vs the straightforward version): instead
of materializing xn and then qkv = W @ xn, compute Z = W @ x on raw x
(starts as soon as x lands, no stats dependency) and fold the LN affine
into each PSUM accumulation group:
  q[o,l] = isd_l*Z[o,l] - (mu_l/sd_l)*r[o] + b[o]
         = isd_l * ( Z[o,l] + (-r[o]*mu_l + b[o]*sd_l) )
where r[o] = sum_c W[o,c]. The parenthesized correction is rank-2 ->
one K=2 matmul per group (lhsT = [-r; b] from host, rhs = [mu; sd] rows
from the on-device stats), and the isd_l scale rides the PSUM->SBUF
drain (one DVE tensor_tensor with a broadcast isd tile).

On-chip layout (per core):
  x:      [C=512, L=1024] as 4 tiles of [128, 1024] (C on partitions)
  LN stats via ones-matmul column sums; [128, 8]-layout scalar chain.
  q, k:   [512, L] 4 tiles [128, 1024]
  vT:     [L, 512] 8 tiles [128, 1024] fp16: [64 v | 64 ones] per head
          so one [128,128] stationary computes AV (rows 0-63) and the
          softmax denominator broadcast (rows 64-127) in a single matmul.
  scores: computed transposed, S^T[j, i], per head pair (row-packed
          K=64 matmuls via tile_position); exp on ACT (scale=1/8) out of
          PSUM into fp16 P^T tiles; causal mask applied post-exp as a
          0/1 triangular multiply on the diagonal 128-col block (GpSimd).
  attention pairs are software-pipelined (scores(p+1) ahead of AV(p)).
  proj:   [512, 512] @ O.
"""

import numpy as np

import concourse.bass as bass
import concourse.mybir as mybir
import concourse.tile as tile
from concourse import bacc
from concourse.bass import ds, ts
from concourse.bass_utils import run_bass_kernel_spmd


F32 = mybir.dt.float32
FP16 = mybir.dt.float16

B, C, H, W = 8, 512, 32, 32
L = H * W                      # 1024
HEADS = 8
DM = 512
DH = 64                        # d_head
EPS = 1e-5
NCORES = 8

# scores^T chunking per j-tile t: list of (i_start, n_cols); each chunk
# stays inside one 512-col PSUM bank of the per-head mega region.
ST_CHUNKS = {
    0: [(0, 512), (512, 512)],
    1: [(128, 512), (640, 384)],
    2: [(256, 512), (768, 256)],
    3: [(384, 512), (896, 128)],
    4: [(512, 512)],
    5: [(640, 384)],
    6: [(768, 256)],
    7: [(896, 128)],
}
ST_EXT = {t: chunks[-1][0] + chunks[-1][1] - 128 * t for t, chunks in ST_CHUNKS.items()}


def _emit(nc, tc):
    x_d = nc.dram_tensor("x", [C, L], FP16, kind="ExternalInput").ap()
    wqkvT_d = nc.dram_tensor("wqkvT", [C, 3 * DM], FP16, kind="ExternalInput").ap()
    wprojT_d = nc.dram_tensor("wprojT", [DM, C], FP16, kind="ExternalInput").ap()
    fix_d = nc.dram_tensor("fix", [2, 3 * DM], FP16, kind="ExternalInput").ap()
    onescol_d = nc.dram_tensor("onescol", [128, 1], FP16, kind="ExternalInput").ap()
    onesrow_d = nc.dram_tensor("onesrow", [1, 128], FP16, kind="ExternalInput").ap()
    bproj_d = nc.dram_tensor("bproj", [C], F32, kind="ExternalInput").ap()
    y_d = nc.dram_tensor("y", [C, L], F32, kind="ExternalOutput").ap()

    fexp = mybir.ActivationFunctionType.Exp
    fsqrt = mybir.ActivationFunctionType.Sqrt
    fcopy = mybir.ActivationFunctionType.Copy

    with (
        tc.tile_pool(name="const", bufs=1) as cpool,
        tc.tile_pool(name="pers", bufs=1) as pers,
        tc.tile_pool(name="pT", bufs=17) as ppool,
    ):
        # ======== persistent tiles ========
        q_t = [pers.tile([128, L], FP16, tag=f"q{m}", name=f"q{m}") for m in range(4)]
        k_t = [pers.tile([128, L], FP16, tag=f"k{m}", name=f"k{m}") for m in range(4)]
        vT_t = [pers.tile([128, 2 * DM], FP16, tag=f"vT{m}", name=f"vT{m}") for m in range(8)]
        o_t = [pers.tile([128, L], FP16, tag=f"o{m}", name=f"o{m}") for m in range(4)]
        wproj_t = [pers.tile([128, C], FP16, tag=f"wp{m}", name=f"wp{m}") for m in range(4)]
        x_t = [pers.tile([128, L], FP16, tag=f"x{c}", name=f"x{c}") for c in range(4)]
        w_t = [pers.tile([128, 3 * DM], FP16, tag=f"w{c}", name=f"w{c}") for c in range(4)]
        fix_t = pers.tile([2, 3 * DM], FP16, tag="fix", name="fix")
        bp4 = pers.tile([128, 4], F32, tag="bp4", name="bp4")
        musd = pers.tile([2, L], FP16, tag="musd", name="musd")
        s_row = pers.tile([1, L], FP16, tag="srow", name="srow")
        t_row = pers.tile([1, L], FP16, tag="trow", name="trow")
        bs_t = pers.tile([128, L], FP16, tag="bs", name="bs")

        # ======== input DMAs, spread across queues ========
        ones_col = cpool.tile([128, 1], FP16, tag="ones_col")
        ones_row = cpool.tile([1, 128], FP16, tag="ones_row")
        eps128 = cpool.tile([128, 1], F32, tag="eps")
        tri = cpool.tile([128, 128], FP16, tag="tri")

        nc.sync.dma_start(x_t[0][:], x_d[ts(0, 128), :])
        nc.gpsimd.dma_start(x_t[1][:], x_d[ts(1, 128), :])
        nc.sync.dma_start(x_t[2][:], x_d[ts(2, 128), :])
        nc.gpsimd.dma_start(x_t[3][:], x_d[ts(3, 128), :])
        nc.sync.dma_start(ones_col[:], onescol_d[:])
        nc.sync.dma_start(ones_row[:], onesrow_d[:])
        for c in range(4):
            nc.scalar.dma_start(w_t[c][:], wqkvT_d[ts(c, 128), :])
        nc.scalar.dma_start(fix_t[:], fix_d[:])
        nc.gpsimd.dma_start(bp4[:], bproj_d[:].rearrange("(o p) -> p o", p=128))
        for m in range(4):
            nc.scalar.dma_start(wproj_t[m][:], wprojT_d[ts(m, 128), :])

        nc.gpsimd.memset(tri[:], 1.0)
        # tri[p, f] = 1.0 if f >= p else 0.0   (keep i_rel >= j_rel)
        nc.gpsimd.affine_select(
            out=tri[:], in_=tri[:],
            compare_op=mybir.AluOpType.is_ge,
            fill=0.0, base=0, pattern=[[1, 128]], channel_multiplier=-1,
        )
        # vT ones columns: strided memset of the odd 64-col groups only
        for m8 in range(8):
            nc.gpsimd.memset(
                vT_t[m8][:, :].rearrange("p (h o) -> p h o", o=128)[:, :, ds(64, 64)],
                1.0,
            )
        garb = cpool.tile([128, 512], FP16, tag="garb")
        nc.vector.memset(garb[:], 0.0)
        nc.vector.memset(eps128[:], EPS)

        # ======== stats + qkv head phase ========
        # PSUM plan (8 banks, stack):
        #   psA01 (banks 0-1, bufs=2, never released early): bcast ->
        #     v-groups -> AV -> proj, all [128,512] tiles recycling the
        #     same two banks with natural deps.
        #   psHead (banks 2-7): stat bounce (2 banks, freed early) + 4 qkv
        #     group slots. Released after qkv emission; the attention megas
        #     (psT, 6 banks) land on these banks -- the first megas alias
        #     the early-freed stat banks so the exp stream starts early.
        psA01 = tc.alloc_tile_pool(name="psA01", bufs=2, space="PSUM")
        # size psA01 before psHead's layout is computed (stack allocator)
        bc_tiles = [psA01.tile([128, 512], F32, tag="mm2", name=f"bc{ch}")
                    for ch in range(2)]
        psHead = tc.alloc_tile_pool(name="psHead", bufs=4, space="PSUM")

        # PE warmup bridging the DMA head: garbage matmuls keep the HAM
        # clock-gate open so the real stream runs at 2.4 GHz.
        wu = psHead.tile([128, 512], F32, tag="mm", name="wu")
        for _ in range(20):
            nc.tensor.matmul(wu[:], garb[:, ds(0, 128)], garb[:],
                             start=True, stop=True)

        # column sums of x and x^2 -> [1, 1024] stats psum (slot reused)
        sq_t = []
        for c in range(4):
            sq = pers.tile([128, L], FP16, tag=f"sq{c}", name=f"sq{c}")
            nc.vector.tensor_mul(sq[:], x_t[c][:], x_t[c][:])
            sq_t.append(sq)
        stats_sb = pers.tile([1, 2048], F32, tag="statsb", name="stats_sb")
        stat_s = psHead.tile([1, 1024], F32, tag="stat", bufs=1, name="stat_s")
        for ch in range(2):
            for c in range(4):
                nc.tensor.matmul(
                    stat_s[:, ds(512 * ch, 512)], ones_col[:],
                    x_t[c][:, ts(ch, 512)],
                    start=(c == 0), stop=(c == 3),
                )
        nc.scalar.activation(stats_sb[:, ds(0, 1024)], stat_s[:],
                             fcopy, scale=1.0 / C)
        stat_q = psHead.tile([1, 1024], F32, tag="stat", bufs=1, name="stat_q")
        for ch in range(2):
            for c in range(4):
                nc.tensor.matmul(
                    stat_q[:, ds(512 * ch, 512)], ones_col[:],
                    sq_t[c][:, ts(ch, 512)],
                    start=(c == 0), stop=(c == 3),
                )
        nc.scalar.activation(stats_sb[:, ds(1024, 1024)], stat_q[:],
                             fcopy, scale=1.0 / C)

        # stats chain in [128, 8] layout (1-partition DVE ops are ~100x
        # slower per element; bounce via ACT copy + SBUF DMA reshape).
        st = pers.tile([128, 16], F32, tag="st", name="st")   # mu cols 0-7, msq 8-15
        nc.sync.dma_start(st[ds(0, 64), ds(0, 8)], stats_sb[:, ds(0, 512)])
        nc.sync.dma_start(st[ds(64, 64), ds(0, 8)], stats_sb[:, ds(512, 512)])
        nc.scalar.dma_start(st[ds(0, 64), ds(8, 8)], stats_sb[:, ds(1024, 512)])
        nc.scalar.dma_start(st[ds(64, 64), ds(8, 8)], stats_sb[:, ds(1536, 512)])
        mu2 = pers.tile([128, 8], F32, tag="mu2", name="mu2")
        nc.vector.tensor_mul(mu2[:], st[:, ds(0, 8)], st[:, ds(0, 8)])
        nc.vector.tensor_sub(mu2[:], st[:, ds(8, 8)], mu2[:])       # var
        # 1/sd via ln+exp: same ACT table set as the attention exps, so the
        # whole kernel needs no sqrt-set reload. sd = (var+eps)*isd.
        lnv = pers.tile([128, 8], F32, tag="lnv", name="lnv")
        nc.scalar.activation(lnv[:], mu2[:], mybir.ActivationFunctionType.Ln,
                             bias=eps128[:])
        isd_t = pers.tile([128, 8], F32, tag="isd", name="isd")
        nc.scalar.activation(isd_t[:], lnv[:], fexp, scale=-0.5)
        sd_t = pers.tile([128, 8], F32, tag="sd", name="sd")
        nc.vector.scalar_tensor_tensor(sd_t[:], mu2[:], EPS, isd_t[:],
                                       mybir.AluOpType.add,
                                       mybir.AluOpType.mult)
        t_t = pers.tile([128, 8], F32, tag="tt", name="t_t")
        nc.vector.tensor_mul(t_t[:], st[:, ds(0, 8)], isd_t[:])
        s16 = pers.tile([128, 32], FP16, tag="s16", name="s16")
        nc.vector.tensor_copy(s16[:, ds(0, 8)], st[:, ds(0, 8)])    # mu
        nc.vector.tensor_copy(s16[:, ds(8, 8)], sd_t[:])            # sd
        nc.vector.tensor_copy(s16[:, ds(16, 8)], isd_t[:])          # 1/sd
        nc.vector.tensor_copy(s16[:, ds(24, 8)], t_t[:])            # mu/sd
        nc.sync.dma_start(musd[ds(0, 1), :], s16[:, ds(0, 8)])
        nc.scalar.dma_start(musd[ds(1, 1), :], s16[:, ds(8, 8)])
        nc.sync.dma_start(s_row[ds(0, 1), :], s16[:, ds(16, 8)])
        nc.scalar.dma_start(t_row[ds(0, 1), :], s16[:, ds(24, 8)])

        # broadcast isd down 128 partitions via K=1 matmul
        for ch in range(2):
            bc_ps = bc_tiles[ch]
            nc.tensor.matmul(bc_ps[:], ones_row[:],
                             s_row[:, ts(ch, 512)], start=True, stop=True)
            nc.vector.tensor_copy(bs_t[:, ts(ch, 512)], bc_ps[:])

        # ---- qkv groups: Z = W @ x  (+ K=2 LN/bias fixup), drain scaled ----
        def qk_group(m, name_off, dst, pool):
            for ch in range(2):
                ps = pool.tile([128, 512], F32, tag="mm")
                for c in range(4):
                    nc.tensor.matmul(
                        ps[:], w_t[c][:, ds(name_off + m * 128, 128)],
                        x_t[c][:, ts(ch, 512)],
                        start=(c == 0), stop=False,
                    )
                nc.tensor.matmul(
                    ps[:], fix_t[:, ds(name_off + m * 128, 128)],
                    musd[:, ts(ch, 512)],
                    start=False, stop=True,
                )
                nc.vector.tensor_mul(dst[:, ts(ch, 512)], ps[:], bs_t[:, ts(ch, 512)])

        def v_group(m8, pool):
            # lhsT is xs = x*isd (recycled sq tiles), so v comes out already
            # scaled; the K=1 fixup adds -t_l*r_v[o]; drain is a plain
            # strided cast into the [64 v | 64 ones] head slots.
            ps = pool.tile([128, 512], F32, tag="mm2")
            for c in range(4):
                nc.tensor.matmul(
                    ps[:], sq_t[c][:, ts(m8, 128)], w_t[c][:, ds(2 * DM, DM)],
                    start=(c == 0), stop=False,
                )
            nc.tensor.matmul(
                ps[:], t_row[ds(0, 1), ts(m8, 128)], fix_t[ds(0, 1), ds(2 * DM, DM)],
                start=False, stop=True,
            )
            nc.vector.tensor_copy(
                vT_t[m8][:, :].rearrange("p (h o) -> p h o", o=128)[:, :, ds(0, 64)],
                ps[:, :].rearrange("p (h o) -> p h o", o=64),
            )

        qk_group(0, 0, q_t[0], psHead)
        qk_group(0, DM, k_t[0], psHead)
        # priority mark: attention instructions are later re-prioritized to
        # land here so the scheduler interleaves them with the rest of qkv
        p_mark = tc.cur_priority

        for m in (1, 2, 3):
            qk_group(m, 0, q_t[m], psHead)
            qk_group(m, DM, k_t[m], psHead)

        # xs = x * isd, recycling the sq tiles (stats already consumed them)
        for c in range(4):
            nc.vector.tensor_mul(sq_t[c][:], x_t[c][:], bs_t[:])
        for m8 in range(8):
            v_group(m8, psA01)

        psHead.release()

        # ======== attention (software-pipelined with remaining qkv) ========
        pT_pairs = {}

        def emit_scores(p, psT):
            pT_tiles = {}
            for t in range(8):
                ext = ST_EXT[t]
                i0 = 128 * t
                pT = ppool.tile([128, 2048], FP16, tag="pT")
                megas = []
                for hh in range(2):
                    megas.append(psT.tile([128, 1024], F32, tag="sT",
                                          name=f"sT{p}_{t}_{hh}"))
                for (ist, ncols) in ST_CHUNKS[t]:
                    for hh in range(2):
                        pb = 64 * hh
                        nc.tensor.matmul(
                            megas[hh][:, ds(ist - i0, ncols)],
                            k_t[p][ds(pb, 64), ts(t, 128)],
                            q_t[p][ds(pb, 64), ds(ist, ncols)],
                            start=True, stop=True,
                            tile_position=(pb, 0),
                        )
                for hh in range(2):
                    nc.scalar.activation(
                        pT[:, ds(hh * 1024, ext)],
                        megas[hh][:, ds(0, ext)],
                        fexp, scale=0.125,
                    )
                    nc.gpsimd.tensor_mul(
                        pT[:, ds(hh * 1024, 128)], pT[:, ds(hh * 1024, 128)], tri[:]
                    )
                pT_tiles[t] = pT
            pT_pairs[p] = pT_tiles

        def emit_av(p, psAV, rsb):
            pT_tiles = pT_pairs.pop(p)
            for cch in range(2):
                tlist = range(4) if cch == 0 else range(8)
                avs = []
                for hh in range(2):
                    h = 2 * p + hh
                    av = psAV.tile([128, 512], F32, tag="mm2",
                                   name=f"av{p}_{cch}_{hh}")
                    avs.append(av)
                    for ti, t in enumerate(tlist):
                        lo = max(cch * 512, 128 * t)
                        n = (cch + 1) * 512 - lo
                        nc.tensor.matmul(
                            av[:, ds(lo - cch * 512, n)],
                            vT_t[t][:, ds(128 * h, 128)],
                            pT_tiles[t][:, ds(hh * 1024 + lo - 128 * t, n)],
                            start=(ti == 0), stop=(ti == len(tlist) - 1),
                        )
                for hh in range(2):
                    rec = rsb.tile([128, 512], F32, tag="rec")
                    nc.vector.reciprocal_approx_fast(rec[:], avs[hh][:, :])
                    nc.vector.tensor_mul(
                        o_t[p][ds(64 * hh, 64), ts(cch, 512)],
                        avs[hh][ds(0, 64), :], rec[ds(64, 64), :],
                    )

        fident = mybir.ActivationFunctionType.Identity
        with (
            tc.tile_pool(name="psT", bufs=3, space="PSUM") as psT,
            tc.tile_pool(name="rsb", bufs=2) as rsb,
            tc.high_priority(offset=tc.cur_priority - p_mark),
        ):
            emit_scores(0, psT)
            emit_scores(1, psT)
            emit_av(0, psA01, rsb)
            emit_scores(2, psT)
            emit_av(1, psA01, rsb)
            emit_scores(3, psT)
            emit_av(2, psA01, rsb)
            emit_av(3, psA01, rsb)

            # ======== output projection ========
            # ch-major: the ch=0 half only needs the cch0 AV drains of all
            # pairs, so it runs under the last pair's cch1 window. Bias-add
            # drains on ACT (idle after the exps).
            yq = [nc.sync, nc.gpsimd, nc.scalar, nc.sync]
            for ch in range(2):
                for m in range(4):
                    yt = rsb.tile([128, 512], F32, tag="y")
                    ps = psA01.tile([128, 512], F32, tag="mm2")
                    for c2 in range(4):
                        nc.tensor.matmul(
                            ps[:], wproj_t[c2][:, ts(m, 128)],
                            o_t[c2][:, ts(ch, 512)],
                            start=(c2 == 0), stop=(c2 == 3),
                        )
                    nc.scalar.activation(yt[:], ps[:], fident,
                                         bias=bp4[:, ds(m, 1)])
                    yq[m].dma_start(y_d[ts(m, 128), ts(ch, 512)], yt[:])
        psA01.release()


_NC_CACHE = None


def build_nc():
    global _NC_CACHE
    if _NC_CACHE is None:
        nc = bacc.Bacc("TRN2", target_bir_lowering=False, debug=False)
        with tile.TileContext(nc) as tc:
            _emit(nc, tc)
        nc.compile()
        _NC_CACHE = nc
    return _NC_CACHE


def host_inputs(x, ln_g, ln_b, qkv_w, qkv_b, proj_w, proj_b, pos_h, pos_w):
    """Fold LN affine + biases; build per-core input maps."""
    x = np.asarray(x, np.float32)
    ln_g = np.asarray(ln_g, np.float32)
    ln_b = np.asarray(ln_b, np.float32)
    qkv_w = np.asarray(qkv_w, np.float32)
    qkv_b = np.asarray(qkv_b, np.float32)
    proj_w = np.asarray(proj_w, np.float32)
    proj_b = np.asarray(proj_b, np.float32)

    w_eff = qkv_w * ln_g[None, :]                    # [1536, 512]
    b_eff = qkv_b + qkv_w @ ln_b                     # [1536]
    wqkvT = np.ascontiguousarray(w_eff.T)            # [512, 1536]
    bq, bv = b_eff[:DM], b_eff[2 * DM:]
    bproj = proj_b + proj_w @ bv                     # [512]
    wprojT = np.ascontiguousarray(proj_w.T)          # [512, 512]

    r = w_eff.sum(axis=1)                            # [1536] row sums
    fix = np.zeros((2, 3 * DM), np.float32)
    fix[0, :] = -r
    fix[1, :DM] = bq                                 # k-bias dropped, v folded

    common = {
        "wqkvT": wqkvT.astype(np.float16),
        "wprojT": wprojT.astype(np.float16),
        "fix": fix.astype(np.float16),
        "bproj": np.ascontiguousarray(bproj),
        "onescol": np.ones((128, 1), np.float16),
        "onesrow": np.ones((1, 128), np.float16),
    }
    in_maps = []
    for b in range(B):
        m = dict(common)
        m["x"] = np.ascontiguousarray(x[b].reshape(C, L)).astype(np.float16)
        in_maps.append(m)
    return in_maps


def kernel(x, ln_g, ln_b, qkv_w, qkv_b, proj_w, proj_b, pos_h, pos_w, **kw):
    nc = build_nc()
    in_maps = host_inputs(x, ln_g, ln_b, qkv_w, qkv_b, proj_w, proj_b, pos_h, pos_w)
    res = run_bass_kernel_spmd(nc, in_maps, core_ids=list(range(NCORES)))
    out = np.stack([res.results[b]["y"].reshape(C, H, W) for b in range(B)])
    return out.astype(np.float32)


if __name__ == "__main__":
    nc = build_nc()
    print("built + compiled ok")
